# revision 35
# baseline (speedup 1.0000x reference)
"""Conv2d 3x3 (stride 1, pad 1) Trainium2 Bass kernel.

Problem: x (32, 128, 56, 56) fp32, kernels (256, 128, 3, 3) fp32, b (256,) fp32
-> out (32, 256, 56, 56) fp32.

Strategy:
  - Data-parallel over batch: 32 images / 8 cores = 4 images per core. SPMD,
    no collectives.
  - Per core: contraction dim C_in=128 lives on SBUF partitions. The 3x3 conv
    is 9 shifted [128c_in x 128c_out x <=448] matmuls accumulated in PSUM (one
    per kernel tap). Zero padding is implicit: boundary taps write a ragged
    sub-window of the PSUM tile (3D out APs) and simply skip the rows/cols a
    zero pad would have contributed to; the center tap goes first and writes
    the full window with start=True so every element is initialized.
  - Output tiled as [c_out half (128 partitions), 8 rows x 56 cols = 448 free]
    (<= 512 fp32, one PSUM bank). 2 halves x 7 row blocks x 4 images = 56
    accumulation groups of 9 matmuls each per core.
  - Inputs cast to bf16 on host (PE runs 2x the fp32 rate; accumulation stays
    fp32 in PSUM). Bias added during PSUM->SBUF eviction on ScalarE.
  - Startup choreography (the matmul work itself is within ~3% of the PE
    roofline, so the wins are in the first/last 15us): x loaded as paced
    row-chunks, weights split over all three DMA queues' early slots, and a
    6-matmul dummy warm-up flips the HAM clock gate to 2.4 GHz before the
    first real matmul issues.
"""

import numpy as np
import ml_dtypes

import concourse.bass as bass
import concourse.tile as tile
from concourse import bacc, mybir
from concourse.bass_utils import run_bass_kernel_spmd

N_CORES = 8
N_FULL = 32
N_PER = N_FULL // N_CORES  # 4 images per core
C_IN = 128
C_OUT = 256
H = W = 56
HW = H * W
KS = 3
R = 8              # output rows per matmul group
NB = H // R        # 7 row blocks
NFREE = R * W      # 448 <= 512 (one PSUM bank of fp32)

_DT = mybir.dt.bfloat16

# Tap order: center tap (kh=1, kw=1) first — it writes the FULL [128, 448]
# window, so start=True initializes every PSUM element and the ragged
# boundary taps can accumulate into sub-windows. Host-side weight layout
# follows this order so the first weight-DMA part covers the first taps.
TAPS = [(1, 1)] + [
    (kh, kw) for kh in range(KS) for kw in range(KS) if not (kh == 1 and kw == 1)
]
# Weight DMA parts as (tap_lo, tap_hi) ranges: part 0 covers the first taps
# so the first matmuls wait only on it; later parts overlap the first groups.
WPARTS = [(0, 1), (1, 3), (3, 6), (6, 9)]
# issue queue per part: parts alternate between the gpsimd and scalar DMA
# queues so they complete in parallel (~2 transfers per queue)
# part2 is emitted on sync between chunk0 and chunk1 (see _build); each
# queue's k-th DMA completes ~1.3us later than its (k-1)-th, so every
# startup-critical transfer gets an early slot on some queue.
_WPART_ENG = ["scalar", "gpsimd", None, "gpsimd"]


def _build():
    nc = bacc.Bacc(
        "TRN2",
        target_bir_lowering=False,
        debug=False,
        num_devices=N_CORES,
    )
    xs = nc.dram_tensor("xs", [N_PER, C_IN, H, W], _DT, kind="ExternalInput").ap()
    wt = nc.dram_tensor("wt", [C_IN, KS * KS * C_OUT], _DT, kind="ExternalInput").ap()
    bt = nc.dram_tensor("bt", [128, 2], mybir.dt.float32, kind="ExternalInput").ap()
    y = nc.dram_tensor(
        "y", [N_PER, C_OUT, HW], mybir.dt.float32, kind="ExternalOutput"
    ).ap()

    with tile.TileContext(nc) as tc:
        with (
            tc.tile_pool(name="const", bufs=1) as const,
            tc.tile_pool(name="wpool", bufs=1, space="PSUM") as wpool,
            tc.tile_pool(name="xpool", bufs=N_PER * NB) as xpool,
            tc.tile_pool(name="pspool", bufs=7, space="PSUM") as pspool,
            tc.tile_pool(name="opool", bufs=4) as opool,
        ):
            # PE warm-up: dummy matmuls on a zeroed scratch tile depend on no
            # DMA, so they run during the input-load window and lift the HAM
            # clock gate (1.2 -> 2.4 GHz) before real matmuls arrive.
            warm = const.tile([128, 512], _DT)
            nc.vector.memset(warm[:], 0.0)
            wps = wpool.tile([128, 512], mybir.dt.float32)
            N_WARM = 5
            for i in range(N_WARM):
                nc.tensor.matmul(
                    wps[:],
                    lhsT=warm[:, :128],
                    rhs=warm[:],
                    start=(i == 0),
                    stop=(i == N_WARM - 1),
                )

            # weights/bias split across the gpsimd and scalar DMA queues
            # (sync carries the x chunks) so all parts land in parallel by
            # ~11us; separate tiles per part so the first matmuls wait only
            # on the part they read
            wparts = []
            for p, (lo, hi) in enumerate(WPARTS):
                wp_sb = const.tile(
                    [C_IN, (hi - lo) * C_OUT], _DT, name=f"wt_sb{p}"
                )
                if _WPART_ENG[p] is not None:
                    eng = getattr(nc, _WPART_ENG[p])
                    eng.dma_start(
                        out=wp_sb[:], in_=wt[:, lo * C_OUT : hi * C_OUT]
                    )
                wparts.append(wp_sb)
            bias_sb = const.tile([128, 2], mybir.dt.float32)
            nc.scalar.dma_start(out=bias_sb[:], in_=bt)

            taps = TAPS
            for n in range(N_PER):
                # 7 row-chunks per image with 1-row halos: chunk c holds input
                # rows (8c-1 .. 8c+9), i.e. tile row t <-> input row 8c-1+t.
                # Chunks are emitted paced — two ahead of the consuming group —
                # so startup HBM bandwidth goes to the weights, not to chunk
                # loads that aren't needed for another 10us.
                chunks = [None] * NB

                def load_chunk(c, n=n, chunks=chunks):
                    xc = xpool.tile([C_IN, R + 2, W], _DT, tag="xc", name=f"xc{n}_{c}")
                    lo = max(0, c * R - 1)
                    hi = min(H, c * R + R + 1)
                    nc.sync.dma_start(
                        out=xc[:, lo - (c * R - 1) : hi - (c * R - 1), :],
                        in_=xs[n, :, lo:hi, :],
                    )
                    chunks[c] = xc

                load_chunk(0)
                if n == 0:
                    # part2 weights take sync's 2nd queue slot (needed before
                    # chunk 1): see _WPART_ENG comment.
                    lo2, hi2 = WPARTS[2]
                    nc.sync.dma_start(
                        out=wparts[2][:], in_=wt[:, lo2 * C_OUT : hi2 * C_OUT]
                    )
                load_chunk(1)
                for half in range(2):
                    for rb in range(NB):
                        if half == 0 and rb + 2 < NB:
                            load_chunk(rb + 2)
                        xc = chunks[rb]
                        ps = pspool.tile([128, NFREE], mybir.dt.float32, tag="ps")
                        ps3 = ps[:].rearrange("p (r c) -> p r c", r=R)
                        r0 = rb * R
                        for idx, (kh, kw) in enumerate(taps):
                            dh, dw = kh - 1, kw - 1
                            rlo = max(r0, -dh)
                            rhi = min(r0 + R, H - dh)
                            clo = max(0, -dw)
                            chi = min(W, W - dw)
                            p = next(
                                i for i, (lo, hi) in enumerate(WPARTS) if idx < hi
                            )
                            off = (idx - WPARTS[p][0]) * C_OUT + half * 128
                            nc.tensor.matmul(
                                ps3[:, rlo - r0 : rhi - r0, clo:chi],
                                lhsT=wparts[p][:, off : off + 128],
                                rhs=xc[
                                    :,
                                    rlo + dh - r0 + 1 : rhi + dh - r0 + 1,
                                    clo + dw : chi + dw,
                                ],
                                start=(idx == 0),
                                stop=(idx == len(taps) - 1),
                            )
                        ot = opool.tile([128, NFREE], mybir.dt.float32, tag="ot")
                        nc.scalar.activation(
                            ot[:],
                            ps[:],
                            mybir.ActivationFunctionType.Identity,
                            bias=bias_sb[:, half : half + 1],
                            scale=1.0,
                        )
                        y_slice = y[
                            n,
                            half * 128 : (half + 1) * 128,
                            rb * NFREE : (rb + 1) * NFREE,
                        ]
                        last = (
                            n == N_PER - 1 and half == 1 and rb == NB - 1
                        )
                        if last:
                            # final store gates the end-of-kernel barrier;
                            # split across two queues to halve its drain time
                            hf = NFREE // 2
                            nc.sync.dma_start(
                                out=y_slice[:, :hf], in_=ot[:, :hf]
                            )
                            nc.scalar.dma_start(
                                out=y_slice[:, hf:], in_=ot[:, hf:]
                            )
                        else:
                            nc.sync.dma_start(out=y_slice, in_=ot[:])
    nc.compile()
    return nc


_NC = None


def _get_nc():
    global _NC
    if _NC is None:
        _NC = _build()
    return _NC


def _prep_inputs(x, kernels, b):
    bf16 = ml_dtypes.bfloat16
    xb = np.ascontiguousarray(x, dtype=np.float32).astype(bf16)
    # [O, I, kh, kw] -> [I, tap, O] in TAPS order -> [128, 9*256]
    wk = np.transpose(np.asarray(kernels, dtype=np.float32), (1, 2, 3, 0))
    wtb = np.ascontiguousarray(
        np.stack([wk[:, kh, kw, :] for kh, kw in TAPS], axis=1)
    ).reshape(C_IN, KS * KS * C_OUT).astype(bf16)
    # bias [256] -> [128, 2]: column h holds b[h*128 : (h+1)*128]
    btb = np.ascontiguousarray(
        np.asarray(b, dtype=np.float32).reshape(2, 128).T
    )
    return xb, wtb, btb


def kernel(x, kernels, b):
    nc = _get_nc()
    xb, wtb, btb = _prep_inputs(x, kernels, b)
    in_maps = [
        {"xs": xb[i * N_PER : (i + 1) * N_PER], "wt": wtb, "bt": btb}
        for i in range(N_CORES)
    ]
    res = run_bass_kernel_spmd(nc, in_maps, core_ids=list(range(N_CORES)))
    out = np.concatenate(
        [r["y"].reshape(N_PER, C_OUT, H, W) for r in res.results], axis=0
    )
    return np.ascontiguousarray(out, dtype=np.float32)


# revision 36
# speedup vs baseline: 1.0150x; 1.0150x over previous
"""Conv2d 3x3 (stride 1, pad 1) Trainium2 Bass kernel.

Problem: x (32, 128, 56, 56) fp32, kernels (256, 128, 3, 3) fp32, b (256,) fp32
-> out (32, 256, 56, 56) fp32.

Strategy:
  - Data-parallel over batch: 32 images / 8 cores = 4 images per core. SPMD,
    no collectives.
  - Per core: contraction dim C_in=128 lives on SBUF partitions. The 3x3 conv
    is 9 shifted [128c_in x 128c_out x <=448] matmuls accumulated in PSUM (one
    per kernel tap). Zero padding is implicit: boundary taps write a ragged
    sub-window of the PSUM tile (3D out APs) and simply skip the rows/cols a
    zero pad would have contributed to; the center tap goes first and writes
    the full window with start=True so every element is initialized.
  - Output tiled as [c_out half (128 partitions), 8 rows x 56 cols = 448 free]
    (<= 512 fp32, one PSUM bank). 2 halves x 7 row blocks x 4 images = 56
    accumulation groups of 9 matmuls each per core.
  - Inputs cast to bf16 on host (PE runs 2x the fp32 rate; accumulation stays
    fp32 in PSUM). Bias added during PSUM->SBUF eviction on ScalarE.
  - Startup choreography (the matmul work itself is within ~3% of the PE
    roofline, so the wins are in the first/last 15us): x loaded as paced
    row-chunks, weights split over all three DMA queues' early slots, and a
    6-matmul dummy warm-up flips the HAM clock gate to 2.4 GHz before the
    first real matmul issues.
"""

import numpy as np
import ml_dtypes

import concourse.bass as bass
import concourse.tile as tile
from concourse import bacc, mybir
from concourse.bass_utils import run_bass_kernel_spmd

N_CORES = 8
N_FULL = 32
N_PER = N_FULL // N_CORES  # 4 images per core
C_IN = 128
C_OUT = 256
H = W = 56
HW = H * W
KS = 3
R = 8              # output rows per matmul group
NB = H // R        # 7 row blocks
NFREE = R * W      # 448 <= 512 (one PSUM bank of fp32)

_DT = mybir.dt.bfloat16

# Tap order: center tap (kh=1, kw=1) first — it writes the FULL [128, 448]
# window, so start=True initializes every PSUM element and the ragged
# boundary taps can accumulate into sub-windows. Host-side weight layout
# follows this order so the first weight-DMA part covers the first taps.
TAPS = [(1, 1)] + [
    (kh, kw) for kh in range(KS) for kw in range(KS) if not (kh == 1 and kw == 1)
]
# Weight DMA parts as (tap_lo, tap_hi) ranges: part 0 covers the first taps
# so the first matmuls wait only on it; later parts overlap the first groups.
WPARTS = [(0, 1), (1, 3), (3, 6), (6, 9)]
# issue queue per part: parts alternate between the gpsimd and scalar DMA
# queues so they complete in parallel (~2 transfers per queue)
# part2 is emitted on sync between chunk0 and chunk1 (see _build); each
# queue's k-th DMA completes ~1.3us later than its (k-1)-th, so every
# startup-critical transfer gets an early slot on some queue.
_WPART_ENG = ["scalar", "gpsimd", None, "gpsimd"]


def _build():
    nc = bacc.Bacc(
        "TRN2",
        target_bir_lowering=False,
        debug=False,
        num_devices=N_CORES,
    )
    xs = nc.dram_tensor("xs", [N_PER, C_IN, H, W], _DT, kind="ExternalInput").ap()
    wt = nc.dram_tensor("wt", [C_IN, KS * KS * C_OUT], _DT, kind="ExternalInput").ap()
    bt = nc.dram_tensor("bt", [128, 2], mybir.dt.float32, kind="ExternalInput").ap()
    y = nc.dram_tensor(
        "y", [N_PER, C_OUT, HW], mybir.dt.float32, kind="ExternalOutput"
    ).ap()

    with tile.TileContext(nc) as tc:
        with (
            tc.tile_pool(name="const", bufs=1) as const,
            tc.tile_pool(name="wpool", bufs=1, space="PSUM") as wpool,
            tc.tile_pool(name="xpool", bufs=N_PER * NB) as xpool,
            tc.tile_pool(name="pspool", bufs=7, space="PSUM") as pspool,
            tc.tile_pool(name="opool", bufs=4) as opool,
        ):
            # PE warm-up: dummy matmuls on a zeroed scratch tile depend on no
            # DMA, so they run during the input-load window and lift the HAM
            # clock gate (1.2 -> 2.4 GHz) before real matmuls arrive.
            warm = const.tile([128, 512], _DT)
            nc.vector.memset(warm[:], 0.0)
            wps = wpool.tile([128, 512], mybir.dt.float32)
            N_WARM = 6
            for i in range(N_WARM):
                nc.tensor.matmul(
                    wps[:],
                    lhsT=warm[:, :128],
                    rhs=warm[:],
                    start=(i == 0),
                    stop=(i == N_WARM - 1),
                )

            # weights/bias split across the gpsimd and scalar DMA queues
            # (sync carries the x chunks) so all parts land in parallel by
            # ~11us; separate tiles per part so the first matmuls wait only
            # on the part they read
            wparts = []
            for p, (lo, hi) in enumerate(WPARTS):
                wp_sb = const.tile(
                    [C_IN, (hi - lo) * C_OUT], _DT, name=f"wt_sb{p}"
                )
                if _WPART_ENG[p] is not None:
                    eng = getattr(nc, _WPART_ENG[p])
                    eng.dma_start(
                        out=wp_sb[:], in_=wt[:, lo * C_OUT : hi * C_OUT]
                    )
                wparts.append(wp_sb)
            bias_sb = const.tile([128, 2], mybir.dt.float32)
            nc.scalar.dma_start(out=bias_sb[:], in_=bt)

            taps = TAPS
            for n in range(N_PER):
                # 7 row-chunks per image with 1-row halos: chunk c holds input
                # rows (8c-1 .. 8c+9), i.e. tile row t <-> input row 8c-1+t.
                # Chunks are emitted paced — two ahead of the consuming group —
                # so startup HBM bandwidth goes to the weights, not to chunk
                # loads that aren't needed for another 10us.
                chunks = [None] * NB

                def load_chunk(c, n=n, chunks=chunks):
                    xc = xpool.tile([C_IN, R + 2, W], _DT, tag="xc", name=f"xc{n}_{c}")
                    lo = max(0, c * R - 1)
                    hi = min(H, c * R + R + 1)
                    nc.sync.dma_start(
                        out=xc[:, lo - (c * R - 1) : hi - (c * R - 1), :],
                        in_=xs[n, :, lo:hi, :],
                    )
                    chunks[c] = xc

                load_chunk(0)
                if n == 0:
                    # part2 weights take sync's 2nd queue slot (needed before
                    # chunk 1): see _WPART_ENG comment.
                    lo2, hi2 = WPARTS[2]
                    nc.sync.dma_start(
                        out=wparts[2][:], in_=wt[:, lo2 * C_OUT : hi2 * C_OUT]
                    )
                load_chunk(1)
                for half in range(2):
                    for rb in range(NB):
                        if half == 0 and rb + 2 < NB:
                            load_chunk(rb + 2)
                        xc = chunks[rb]
                        ps = pspool.tile([128, NFREE], mybir.dt.float32, tag="ps")
                        ps3 = ps[:].rearrange("p (r c) -> p r c", r=R)
                        r0 = rb * R
                        for idx, (kh, kw) in enumerate(taps):
                            dh, dw = kh - 1, kw - 1
                            rlo = max(r0, -dh)
                            rhi = min(r0 + R, H - dh)
                            clo = max(0, -dw)
                            chi = min(W, W - dw)
                            p = next(
                                i for i, (lo, hi) in enumerate(WPARTS) if idx < hi
                            )
                            off = (idx - WPARTS[p][0]) * C_OUT + half * 128
                            nc.tensor.matmul(
                                ps3[:, rlo - r0 : rhi - r0, clo:chi],
                                lhsT=wparts[p][:, off : off + 128],
                                rhs=xc[
                                    :,
                                    rlo + dh - r0 + 1 : rhi + dh - r0 + 1,
                                    clo + dw : chi + dw,
                                ],
                                start=(idx == 0),
                                stop=(idx == len(taps) - 1),
                            )
                        ot = opool.tile([128, NFREE], mybir.dt.float32, tag="ot")
                        nc.scalar.activation(
                            ot[:],
                            ps[:],
                            mybir.ActivationFunctionType.Identity,
                            bias=bias_sb[:, half : half + 1],
                            scale=1.0,
                        )
                        nc.sync.dma_start(
                            out=y[
                                n,
                                half * 128 : (half + 1) * 128,
                                rb * NFREE : (rb + 1) * NFREE,
                            ],
                            in_=ot[:],
                        )
    nc.compile()
    return nc


_NC = None


def _get_nc():
    global _NC
    if _NC is None:
        _NC = _build()
    return _NC


def _prep_inputs(x, kernels, b):
    bf16 = ml_dtypes.bfloat16
    xb = np.ascontiguousarray(x, dtype=np.float32).astype(bf16)
    # [O, I, kh, kw] -> [I, tap, O] in TAPS order -> [128, 9*256]
    wk = np.transpose(np.asarray(kernels, dtype=np.float32), (1, 2, 3, 0))
    wtb = np.ascontiguousarray(
        np.stack([wk[:, kh, kw, :] for kh, kw in TAPS], axis=1)
    ).reshape(C_IN, KS * KS * C_OUT).astype(bf16)
    # bias [256] -> [128, 2]: column h holds b[h*128 : (h+1)*128]
    btb = np.ascontiguousarray(
        np.asarray(b, dtype=np.float32).reshape(2, 128).T
    )
    return xb, wtb, btb


def kernel(x, kernels, b):
    nc = _get_nc()
    xb, wtb, btb = _prep_inputs(x, kernels, b)
    in_maps = [
        {"xs": xb[i * N_PER : (i + 1) * N_PER], "wt": wtb, "bt": btb}
        for i in range(N_CORES)
    ]
    res = run_bass_kernel_spmd(nc, in_maps, core_ids=list(range(N_CORES)))
    out = np.concatenate(
        [r["y"].reshape(N_PER, C_OUT, H, W) for r in res.results], axis=0
    )
    return np.ascontiguousarray(out, dtype=np.float32)
